# revision 21
# baseline (speedup 1.0000x reference)
"""Trainium2 Bass kernel for a second-order-CRF (triple-tag) forward loss.

Math (matches the reference):
    flat    = scores.reshape(S, B, T^3)
    tg      = sum_{s,b} flat[s, b, target[s,b]]                    (mask all ones)
    part_2[b,u,v]   = scores[0,b,ST,ST,u] + scores[1,b,ST,u,v]
    part_{t+1}[b,v,w] = logsumexp_u(part_t[b,u,v] + s_t[b,u,v,w])   t=2..S-1
    loss    = (sum_b part_S[b,EN,EN] - tg) / B

Device formulation: exp space with a constant per-step log-offset C
(no per-step log/exp on the serial path):
    D_{t+1}[v,w] = sum_u D_t[u,v] * E_t[u,v,w],   E_t = exp(s_t - C)
and, because the recurrence is LINEAR in exp space, meet-in-the-middle:
    z_b = sum_{i,j} D_64[i,j] * G_64[i,j]
with a backward chain G_t[i,j] = sum_k E_t[i,j,k] * G_{t+1}[j,k] seeded from
G_128 = one-hot(EN,EN).  Host folds the sparse edge steps (0,1 -> D_2 and
127,126 -> G_126), leaving 2x62 dense device steps that run as two
independent serial chains interleaved on the Vector engine.

Per step, per chain, two custom DVE ops (registered per-NEFF):
    TPOSE_ANT      : 32x32-block transpose of the [128,32] state via the
                     DVE reshape front-end (1x, ~210 ns).
    SEGSUMP2X_ANT  : the fused multiply + per-page segmented sum
                     out[p,s] = sum_n X[p,n] * E[p,s,n]
                     running in the 2x_1P packed-bf16 perf mode (both sources
                     read 2 elems/cycle; ~690 ns vs 1244 ns at 1x).  Page
                     sums are written as 32-bit PAIRS via a 5-state
                     page-parity uop FSM (even page sum frozen in stage 4's
                     self-holding flop during the odd page) because lone
                     16-bit subdim writes hang the engine in 2x mode.
Interleaving the two chains hides each op's issue/drain latency:
measured cadence 725 ns per scan step (vs 1244 baseline).

Sharding: batch (32) split 4-per-core across 8 cores.  Host pre-transposes
scores to [s,b,v,w,u] (fwd) / [s,b,v,u,w] (bwd, slot-reversed) and casts to
bf16.  Chunked DMA on the sync+tensor rings; exp(x-C) on ACT (the ~110 us
engine floor that bounds the kernel).  Gold-path gather, seeds, and the
final  z_b = <D_64, G_64^T>;  log / sum  run on host (O(B*T^2)).
"""

import sys

import numpy as np

for _p in ("/opt/trn_rl_repo",):
    if _p not in sys.path:
        sys.path.insert(0, _p)

import copy

import concourse.bass as bass
import concourse.bacc as bacc
import concourse.tile as tile
from concourse import mybir, bass_isa
from concourse import bass_utils
from concourse import dve_ops
from concourse.dve_spec import AluOp, Spec, Src0, Src1, lower, scan
from concourse.dve_uop import (
    DveOpSpec,
    OpConfig,
    OutPath,
    OutSel,
    Trigger,
    TransposeMode,
    UopConfig,
    UopDpConfig,
    AluInp,
    DelayInp,
    InpSel,
)

S = 128          # sequence length
B = 32           # full batch
NCORES = 8
BL = B // NCORES  # batch per core = 4
T = 32           # tag count
P = BL * T       # partitions = 128
START, END = 30, 31
C_OFF = 4.17     # per-step log-space renormalization constant
M = 64           # split point: fwd produces D_64, bwd produces G_64
NSLOT = M - 2    # 62 steps per chain
SB = BL * T * T * T   # element stride between steps (131072)
CHUNKS = [1, 2, 6, 9, 9, 8, 7, 6, 5, 4, 2, 1, 1, 1]
assert sum(CHUNKS) == NSLOT
F32 = mybir.dt.float32
BF16 = mybir.dt.bfloat16
FP8 = mybir.dt.float8e4  # TRN FP8_EXP4 == ml_dtypes.float8_e4m3 (|x| <= 240)

_cache = {}
LAST_RESULT = None  # BassKernelResults of the most recent run (for profiling)


class _Op:
    """Minimal DveOp-alike carrying a fixed pre-built DveOpSpec."""

    def __init__(self, name, spec, subdim, compiled):
        self.name = name
        self.spec = spec
        self.subdim = subdim
        self.perf_en = {}
        self._compiled = compiled

    def compile(self, ver, **_):
        return self._compiled


def _register(name, spec, subdim, uops, uops_2x=None, op_cfg=None, rd1_en=True):
    if name in _cache:
        return _cache[name]
    if name in dve_ops._SUB_OPCODE_FOR_NAME:
        row = dve_ops._SUB_OPCODE_FOR_NAME[name]
    else:
        row = 1 + len(dve_ops.OPS)
        assert row < 0x20, "out of custom DVE opcode rows"
    compiled = DveOpSpec(
        name=name, opcode=row, uops=uops, uops_2x=uops_2x, rd1_en=rd1_en,
        op=op_cfg or OpConfig(),
    )
    compiled.validate("v3")
    op = _Op(name, spec, subdim, compiled)
    if name not in dve_ops._SUB_OPCODE_FOR_NAME:
        dve_ops.OPS.append(op)
        dve_ops._SUB_OPCODE_FOR_NAME[name] = row
        dve_ops.CUSTOM_DVE_SPECS[name] = spec
    _cache[name] = op
    return op


def _segsum_parity_2x_uops():
    """2x_1P packed pair program: pages = 32 outputs, inner = 32 (16 pairs).
    stage0: p_lo = SRC_0*SRC_1; stage1: p_hi = SRC_0_HI*SRC_1_HI (lo saved in
    delay0); stage2: pair = p_hi + p_lo; stage3: running page sum (CURR
    feedback; zero-lane reset on step uops); stage4: even-page sum frozen via
    BYPASS(CURR) self-hold during odd pages, odd running sum riding delay0;
    write (even|odd) as one 32-bit pair at each odd page's subdim-last."""
    def mk(kind):
        u = UopConfig()
        u.enable_input(InpSel.SRC_0, 1)
        u.enable_input(InpSel.SRC_1, 2)
        u.enable_input(InpSel.ZERO, 3)
        u.enable_input(InpSel.SRC_0_HI, 4)
        u.enable_input(InpSel.SRC_1_HI, 5)
        u.datapath_config[0] = (
            UopDpConfig()
            .enable_alu(AluOp.MULTIPLY, AluInp.PREV_DELAY_0, AluInp.PREV_DELAY_1)
            .pass_through_delay(2, 3, 4)
        )
        u.datapath_config[1] = (
            UopDpConfig()
            .enable_alu(AluOp.MULTIPLY, AluInp.PREV_DELAY_3, AluInp.PREV_DELAY_4)
            .enable_delay_from_src(DelayInp.PREV_ALU_OUT, 0)
            .pass_through_delay(2)
        )
        u.datapath_config[2] = (
            UopDpConfig()
            .enable_alu(AluOp.ADD, AluInp.PREV_ALU_OUT, AluInp.PREV_DELAY_0)
            .pass_through_delay(2)
        )
        if kind == "seed":
            u.datapath_config[3] = UopDpConfig().enable_alu(
                AluOp.BYPASS, AluInp.PREV_DELAY_2, AluInp.PREV_DELAY_2
            )
        elif kind in ("stepO", "stepE"):
            u.datapath_config[3] = UopDpConfig().enable_alu(
                AluOp.ADD, AluInp.PREV_DELAY_2, AluInp.PREV_ALU_OUT
            )
        else:  # steadyE / steadyO
            u.datapath_config[3] = UopDpConfig().enable_alu(
                AluOp.ADD, AluInp.CURR_ALU_OUT, AluInp.PREV_ALU_OUT
            )
        hold = kind in ("stepO", "steadyO")
        u.datapath_config[4] = (
            UopDpConfig()
            .enable_alu(
                AluOp.BYPASS,
                AluInp.CURR_ALU_OUT if hold else AluInp.PREV_ALU_OUT,
                AluInp.CURR_ALU_OUT if hold else AluInp.PREV_ALU_OUT,
            )
            .enable_delay_from_src(DelayInp.PREV_ALU_OUT, 0)
        )
        for k in range(5, 8):
            u.datapath_config[k] = (
                UopDpConfig().pass_through_alu().pass_through_delay(0)
            )
        if kind != "seed":
            u.require_inp0 = 1
            u.require_inp1 = 1
        return u

    seed = mk("seed")
    seed.repeat_count = 1
    seed.trigger = (Trigger.COUNT, Trigger.NONE, Trigger.NONE)
    seed.next_uop = (1, 0, 0)

    steadyE = mk("steadyE")
    steadyE.trigger = (Trigger.SRC_TENSOR_DONE, Trigger.SUB_DIM_DONE, Trigger.NONE)
    steadyE.next_uop = (0, 2, 0)

    stepO = mk("stepO")
    stepO.repeat_count = 1
    stepO.trigger = (Trigger.SRC_TENSOR_DONE, Trigger.COUNT, Trigger.NONE)
    stepO.next_uop = (0, 3, 0)

    steadyO = mk("steadyO")
    steadyO.trigger = (Trigger.SRC_TENSOR_DONE, Trigger.SUB_DIM_DONE, Trigger.NONE)
    steadyO.next_uop = (0, 4, 0)
    steadyO.out_last_subdim_enable = 1
    steadyO.enable_output(OutSel.ALU_OUT, OutPath.WR0_LO)
    steadyO.enable_output(OutSel.DELAY_0, OutPath.WR0_HI)

    stepE = mk("stepE")
    stepE.repeat_count = 1
    stepE.trigger = (Trigger.SRC_TENSOR_DONE, Trigger.COUNT, Trigger.NONE)
    stepE.next_uop = (0, 1, 0)

    uops = [seed, steadyE, stepO, steadyO, stepE]
    for u in uops:
        u.validate("v3")
    return uops


def _segsum_parity_1x_uops():
    """Correct 1x fallback (one sum written per page), shaped as 5 states to
    mirror the 2x program (table-gen requires matching state counts)."""
    def mk(kind):
        u = UopConfig()
        u.enable_input(InpSel.SRC_0, 1)
        u.enable_input(InpSel.SRC_1, 2)
        u.enable_input(InpSel.ZERO, 3)
        u.datapath_config[0] = (
            UopDpConfig()
            .enable_alu(AluOp.MULTIPLY, AluInp.PREV_DELAY_0, AluInp.PREV_DELAY_1)
            .pass_through_delay(2)
        )
        if kind == "seed":
            u.datapath_config[1] = UopDpConfig().enable_alu(
                AluOp.BYPASS, AluInp.PREV_DELAY_2, AluInp.PREV_DELAY_2
            )
        elif kind.startswith("step"):
            u.datapath_config[1] = UopDpConfig().enable_alu(
                AluOp.ADD, AluInp.PREV_DELAY_2, AluInp.PREV_ALU_OUT
            )
        else:
            u.datapath_config[1] = UopDpConfig().enable_alu(
                AluOp.ADD, AluInp.CURR_ALU_OUT, AluInp.PREV_ALU_OUT
            )
        for k in range(2, 8):
            u.datapath_config[k] = UopDpConfig().pass_through_alu()
        if kind != "seed":
            u.require_inp0 = 1
            u.require_inp1 = 1
            u.out_last_subdim_enable = 1
            u.enable_output(OutSel.ALU_OUT, OutPath.WR0_LO)
        return u

    seed = mk("seed")
    seed.repeat_count = 1
    seed.trigger = (Trigger.COUNT, Trigger.NONE, Trigger.NONE)
    seed.next_uop = (1, 0, 0)

    steadyE = mk("steadyE")
    steadyE.trigger = (Trigger.SRC_TENSOR_DONE, Trigger.SUB_DIM_DONE, Trigger.NONE)
    steadyE.next_uop = (0, 2, 0)

    stepO = mk("stepO")
    stepO.repeat_count = 1
    stepO.trigger = (Trigger.SRC_TENSOR_DONE, Trigger.COUNT, Trigger.NONE)
    stepO.next_uop = (0, 3, 0)

    steadyO = mk("steadyO")
    steadyO.trigger = (Trigger.SRC_TENSOR_DONE, Trigger.SUB_DIM_DONE, Trigger.NONE)
    steadyO.next_uop = (0, 4, 0)

    stepE = mk("stepE")
    stepE.repeat_count = 1
    stepE.trigger = (Trigger.SRC_TENSOR_DONE, Trigger.COUNT, Trigger.NONE)
    stepE.next_uop = (0, 1, 0)

    uops = [seed, steadyE, stepO, steadyO, stepE]
    for u in uops:
        u.validate("v3")
    return uops


def _tpose_uops():
    """Bare 32x32-block transpose: TRANSPOSE front-end + pass-through body."""
    u = UopConfig()
    u.enable_input(InpSel.SRC_0, 1)
    u.datapath_config[0] = UopDpConfig().enable_alu(
        AluOp.BYPASS, AluInp.PREV_DELAY_0, AluInp.PREV_DELAY_0
    )
    for k in range(1, 8):
        u.datapath_config[k] = UopDpConfig().pass_through_alu()
    u.require_inp0 = 1
    u.trigger = (Trigger.SRC_TENSOR_DONE, Trigger.NONE, Trigger.NONE)
    u.next_uop = (0, 0, 0)
    u.enable_output(OutSel.ALU_OUT, OutPath.WR0_LO)
    u.validate("v3")
    return [u]


def _get_ops():
    if "segsum" in _cache:
        return _cache["segsum"], _cache["tpose"]

    def _ref(in0, in1, s0, s1, imm2):
        return (np.asarray(in0, np.float32) * np.asarray(in1, np.float32)).sum(-1)

    spec = Spec(body=scan(AluOp.ADD, Src0 * Src1), reference=_ref)
    segsum = _register(
        "SEGSUMP2X_ANT", spec, True, _segsum_parity_1x_uops(),
        uops_2x=_segsum_parity_2x_uops(), op_cfg=OpConfig(),
    )
    spec_tp = Spec(body=Src0 + Src0, reference=lambda in0, s0, s1, imm2: in0)
    tpose = _register(
        "TPOSE_ANT", spec_tp, False, _tpose_uops(),
        op_cfg=OpConfig(transpose_mode=TransposeMode.TRANSPOSE), rd1_en=False,
    )
    _cache["segsum"] = segsum
    _cache["tpose"] = tpose
    return segsum, tpose


def _emit_dve(nc, op, *, out, in0, in1=None, perf_max=0):
    """Mirror of bass.Vector._custom_dve, plus the perf_max (byte-36[7:6])
    field that unlocks the 2x_1P table slot."""
    v = nc.vector
    if op.name not in v.bass.m.ant_custom_dve_ops:
        v.bass.m.ant_custom_dve_ops = sorted(
            {*v.bass.m.ant_custom_dve_ops, op.name}
        )
    compiled = op.compile("v3")
    opt = not op.subdim
    in1_elementwise = len(in1.shape) > 2 if in1 is not None else False
    shape = (
        bass_isa.CustomDveShape.STT if in1_elementwise
        else bass_isa.CustomDveShape.TTSS
    )
    isa_opcode = v.bass.isa.Opcode[
        f"NEURON_ISA_TPB_OPCODE_CUSTOM_DVE_ANT_{shape.slot()}"
    ].value
    zero = mybir.ImmediateValue(dtype=mybir.dt.float32, value=0.0)
    ins = [v.lower_ap(in0, for_isa=True, opt=opt)]
    if in1 is not None:
        ins.append(v.lower_ap(in1, for_isa=True, opt=opt))
    ins += [zero, zero]
    outs = [v.lower_ap(out, for_isa=True, opt=opt)]
    return v.add_instruction(
        bass_isa.InstCustomDveAnt(
            name=v.bass.get_next_instruction_name(),
            op_name=op.name,
            rd1_en=compiled.rd1_en,
            subdim=0x02 if op.subdim else 0,
            imm2=0.0,
            shape=shape,
            row=compiled.opcode,
            isa_opcode=isa_opcode,
            ins=ins,
            outs=outs,
            perf_max=perf_max,
        )
    )


def _build_program() -> bass.Bass:
    from contextlib import ExitStack

    segsum, tpose = _get_ops()
    nc = bacc.Bacc("TRN2", target_bir_lowering=False)
    # fwd: steps 2..63 in [s,b,v,w,u]; bwd: steps 125..64 in [s,b,v,u,w]
    scf = nc.dram_tensor("scf", [NSLOT, BL, T, T, T], FP8, kind="ExternalInput")
    scb = nc.dram_tensor("scb", [NSLOT, BL, T, T, T], FP8, kind="ExternalInput")
    d2in = nc.dram_tensor("init_d2", [P, T], BF16, kind="ExternalInput")
    g126in = nc.dram_tensor("init_g126", [P, T], BF16, kind="ExternalInput")
    dout = nc.dram_tensor("dfin", [P, 2 * T], BF16, kind="ExternalOutput")

    with tile.TileContext(nc) as tc, ExitStack() as ctx:
        rawf = ctx.enter_context(tc.tile_pool(name="rawf", bufs=4))
        efp = ctx.enter_context(tc.tile_pool(name="efp", bufs=3))
        spool = ctx.enter_context(tc.tile_pool(name="spool", bufs=3))
        xpool = ctx.enter_context(tc.tile_pool(name="xpool", bufs=4))
        small = ctx.enter_context(tc.tile_pool(name="small", bufs=1))

        cbias = small.tile([P, 1], F32)
        nc.vector.memset(cbias[...], -C_OFF)
        # tiny warm-up activation: forces ACT_TABLE_LOAD at t~8us instead of
        # behind the first chunk's DMA-completion wait
        warm = small.tile([P, 1], F32)
        nc.scalar.activation(
            out=warm[...], in_=cbias[...],
            func=mybir.ActivationFunctionType.Exp,
        )

        def chunk_dma(eng, dst, dram, s0, ch):
            eng.dma_start(
                out=dst[...],
                in_=bass.AP(
                    tensor=dram[...].tensor,
                    offset=s0 * SB,
                    ap=[[T * T, P], [SB, ch], [T, T], [1, T]],
                ),
            )

        # chunk-0 DMAs first: the first exp gates the whole pipeline.
        # Raw/e tiles interleave the two chains per step: [P, ch, 2, T, T]
        # (fwd at [:, :, 0], bwd at [:, :, 1]) -> ONE activation per chunk.
        def chunk_tiles(ch):
            rw = rawf.tile([P, ch, 2, T, T], FP8)
            return rw

        rw0 = chunk_tiles(CHUNKS[0])
        chunk_dma(nc.sync, rw0[:, :, 0], scf, 0, CHUNKS[0])
        chunk_dma(nc.sync, rw0[:, :, 1], scb, 0, CHUNKS[0])

        d0t = small.tile([P, T], BF16)
        d_cur = d0t[...]
        nc.sync.dma_start(out=d_cur, in_=d2in[...])
        g0t = small.tile([P, T], BF16)
        g_cur = g0t[...]
        nc.sync.dma_start(out=g_cur, in_=g126in[...])

        s0 = 0
        for ci, ch in enumerate(CHUNKS):
            if ci == 0:
                rw = rw0
            else:
                rw = chunk_tiles(ch)
                chunk_dma(nc.sync, rw[:, :, 0], scf, s0, ch)
                chunk_dma(nc.sync, rw[:, :, 1], scb, s0, ch)
            ee = efp.tile([P, ch, 2, T, T], BF16)
            if ci == 0:
                # split fwd/bwd so the first fwd step unblocks without
                # waiting for the bwd chunk's DMA
                nc.scalar.activation(
                    out=ee[:, :, 0], in_=rw[:, :, 0],
                    func=mybir.ActivationFunctionType.Exp, bias=cbias[...],
                )
                nc.scalar.activation(
                    out=ee[:, :, 1], in_=rw[:, :, 1],
                    func=mybir.ActivationFunctionType.Exp, bias=cbias[...],
                )
            else:
                nc.scalar.activation(
                    out=ee[...], in_=rw[...],
                    func=mybir.ActivationFunctionType.Exp, bias=cbias[...],
                )
            for j in range(ch):
                last = (s0 + j) == NSLOT - 1
                xf = xpool.tile([P, T], BF16)
                _emit_dve(nc, tpose, out=xf[...], in0=d_cur)
                xb = xpool.tile([P, T], BF16)
                _emit_dve(nc, tpose, out=xb[...], in0=g_cur)
                if last:
                    fin = small.tile([P, 2 * T], BF16)
                    d_nxt, g_nxt = fin[:, 0:T], fin[:, T:2 * T]
                else:
                    dnt = spool.tile([P, T], BF16)
                    gnt = spool.tile([P, T], BF16)
                    d_nxt, g_nxt = dnt[...], gnt[...]
                _emit_dve(
                    nc, segsum, out=d_nxt,
                    in0=xf[...].unsqueeze(1).broadcast_to([P, T, T]),
                    in1=ee[:, j, 0], perf_max=1,
                )
                _emit_dve(
                    nc, segsum, out=g_nxt,
                    in0=xb[...].unsqueeze(1).broadcast_to([P, T, T]),
                    in1=ee[:, j, 1], perf_max=1,
                )
                d_cur, g_cur = d_nxt, g_nxt
            s0 += ch
        nc.sync.dma_start(out=dout[...], in_=fin[...])
    nc.compile()
    return nc


def _get_program() -> bass.Bass:
    if "nc" not in _cache:
        _cache["nc"] = _build_program()
    return _cache["nc"]


def kernel(scores, target, mask=None, **_unused):
    import ml_dtypes

    BH = ml_dtypes.bfloat16
    scores = np.asarray(scores, dtype=np.float32)
    target = np.asarray(target)

    F8 = ml_dtypes.float8_e4m3
    # fwd E-layout [s,b,v,w,u] for steps 2..63; bwd [s,b,v,u,w] for steps
    # 125..64 (slot k = step 125-k)
    scf = np.ascontiguousarray(
        scores[2:M].transpose(0, 1, 3, 4, 2)
    ).astype(F8)
    scb = np.ascontiguousarray(
        scores[M:126].transpose(0, 1, 3, 2, 4)[::-1]
    ).astype(F8)

    # seeds
    p1 = scores[0, :, START, START, :]                    # (B, i)
    part2 = p1[:, :, None] + scores[1, :, START, :, :]    # (B, i, j)
    d2 = np.exp(part2 - C_OFF).astype(BH)                 # stored [(b,i), j]
    g127_j = np.exp(scores[127, :, :, END, END] - C_OFF)  # (B, j)
    g126 = np.exp(scores[126, :, :, :, END] - C_OFF) * g127_j[:, None, :]
    g126 = np.ascontiguousarray(g126.transpose(0, 2, 1)).astype(BH)  # [(b,j), i]

    nc = _get_program()
    in_maps = []
    for core in range(NCORES):
        bs = slice(core * BL, (core + 1) * BL)
        in_maps.append({
            "scf": np.ascontiguousarray(scf[:, bs]),
            "scb": np.ascontiguousarray(scb[:, bs]),
            "init_d2": np.ascontiguousarray(d2[bs]).reshape(P, T),
            "init_g126": np.ascontiguousarray(g126[bs]).reshape(P, T),
        })

    res = bass_utils.run_bass_kernel_spmd(nc, in_maps, core_ids=list(range(NCORES)))
    global LAST_RESULT
    LAST_RESULT = res

    total_z = 0.0
    for core in range(NCORES):
        fin = np.asarray(res.results[core]["dfin"], np.float32).astype(np.float64)
        D = fin[:, 0:T].reshape(BL, T, T)
        G = fin[:, T:2 * T].reshape(BL, T, T)
        z_be = np.einsum("bij,bji->b", D, G)
        total_z += (np.log(z_be) + (S - 1) * C_OFF).sum()

    flat = scores.reshape(S, B, -1)
    tg = np.take_along_axis(flat, target.reshape(S, B, 1).astype(np.int64), axis=2)
    tg_energy = tg.astype(np.float64).sum()

    return np.asarray((total_z - tg_energy) / B, dtype=np.float32)


# revision 23
# speedup vs baseline: 1.0267x; 1.0267x over previous
"""Trainium2 Bass kernel for a second-order-CRF (triple-tag) forward loss.

Math (matches the reference):
    flat    = scores.reshape(S, B, T^3)
    tg      = sum_{s,b} flat[s, b, target[s,b]]                    (mask all ones)
    part_2[b,u,v]   = scores[0,b,ST,ST,u] + scores[1,b,ST,u,v]
    part_{t+1}[b,v,w] = logsumexp_u(part_t[b,u,v] + s_t[b,u,v,w])   t=2..S-1
    loss    = (sum_b part_S[b,EN,EN] - tg) / B

Device formulation: exp space with a constant per-step log-offset C
(no per-step log/exp on the serial path):
    D_{t+1}[v,w] = sum_u D_t[u,v] * E_t[u,v,w],   E_t = exp(s_t - C)
and, because the recurrence is LINEAR in exp space, meet-in-the-middle:
    z_b = sum_{i,j} D_64[i,j] * G_64[i,j]
with a backward chain G_t[i,j] = sum_k E_t[i,j,k] * G_{t+1}[j,k] seeded from
G_128 = one-hot(EN,EN).  Host folds the sparse edge steps (0,1 -> D_2 and
127,126 -> G_126), leaving 2x62 dense device steps that run as two
independent serial chains interleaved on the Vector engine.

Per step, per chain, two custom DVE ops (registered per-NEFF):
    TPOSE_ANT      : 32x32-block transpose of the [128,32] state via the
                     DVE reshape front-end (1x, ~210 ns).
    SEGSUMP2X_ANT  : the fused multiply + per-page segmented sum
                     out[p,s] = sum_n X[p,n] * E[p,s,n]
                     running in the 2x_1P packed-bf16 perf mode (both sources
                     read 2 elems/cycle; ~690 ns vs 1244 ns at 1x).  Page
                     sums are written as 32-bit PAIRS via a 5-state
                     page-parity uop FSM (even page sum frozen in stage 4's
                     self-holding flop during the odd page) because lone
                     16-bit subdim writes hang the engine in 2x mode.
Interleaving the two chains hides each op's issue/drain latency:
measured cadence 725 ns per scan step (vs 1244 baseline).

Sharding: batch (32) split 4-per-core across 8 cores.  Host pre-transposes
scores to [s,b,v,w,u] (fwd) / [s,b,v,u,w] (bwd, slot-reversed) and casts to
bf16.  Chunked DMA on the sync+tensor rings; exp(x-C) on ACT (the ~110 us
engine floor that bounds the kernel).  Gold-path gather, seeds, and the
final  z_b = <D_64, G_64^T>;  log / sum  run on host (O(B*T^2)).
"""

import sys

import numpy as np

for _p in ("/opt/trn_rl_repo",):
    if _p not in sys.path:
        sys.path.insert(0, _p)

import copy

import concourse.bass as bass
import concourse.bacc as bacc
import concourse.tile as tile
from concourse import mybir, bass_isa
from concourse import bass_utils
from concourse import dve_ops
from concourse.dve_spec import AluOp, Spec, Src0, Src1, lower, scan
from concourse.dve_uop import (
    DveOpSpec,
    OpConfig,
    OutPath,
    OutSel,
    Trigger,
    TransposeMode,
    UopConfig,
    UopDpConfig,
    AluInp,
    DelayInp,
    InpSel,
)

S = 128          # sequence length
B = 32           # full batch
NCORES = 8
BL = B // NCORES  # batch per core = 4
T = 32           # tag count
P = BL * T       # partitions = 128
START, END = 30, 31
C_OFF = 4.17     # per-step log-space renormalization constant
M = 64           # split point: fwd produces D_64, bwd produces G_64
NSLOT = M - 2    # 62 steps per chain
SB = BL * T * T * T   # element stride between steps (131072)
CHUNKS = [1, 2, 4, 8, 10, 8, 6, 6, 5, 4, 3, 2, 2, 1]
assert sum(CHUNKS) == NSLOT
F32 = mybir.dt.float32
BF16 = mybir.dt.bfloat16
FP8 = mybir.dt.float8e4  # TRN FP8_EXP4 == ml_dtypes.float8_e4m3 (|x| <= 240)

_cache = {}
LAST_RESULT = None  # BassKernelResults of the most recent run (for profiling)


class _Op:
    """Minimal DveOp-alike carrying a fixed pre-built DveOpSpec."""

    def __init__(self, name, spec, subdim, compiled):
        self.name = name
        self.spec = spec
        self.subdim = subdim
        self.perf_en = {}
        self._compiled = compiled

    def compile(self, ver, **_):
        return self._compiled


def _register(name, spec, subdim, uops, uops_2x=None, op_cfg=None, rd1_en=True):
    if name in _cache:
        return _cache[name]
    if name in dve_ops._SUB_OPCODE_FOR_NAME:
        row = dve_ops._SUB_OPCODE_FOR_NAME[name]
    else:
        row = 1 + len(dve_ops.OPS)
        assert row < 0x20, "out of custom DVE opcode rows"
    compiled = DveOpSpec(
        name=name, opcode=row, uops=uops, uops_2x=uops_2x, rd1_en=rd1_en,
        op=op_cfg or OpConfig(),
    )
    compiled.validate("v3")
    op = _Op(name, spec, subdim, compiled)
    if name not in dve_ops._SUB_OPCODE_FOR_NAME:
        dve_ops.OPS.append(op)
        dve_ops._SUB_OPCODE_FOR_NAME[name] = row
        dve_ops.CUSTOM_DVE_SPECS[name] = spec
    _cache[name] = op
    return op


def _segsum_parity_2x_uops():
    """2x_1P packed pair program: pages = 32 outputs, inner = 32 (16 pairs).
    stage0: p_lo = SRC_0*SRC_1; stage1: p_hi = SRC_0_HI*SRC_1_HI (lo saved in
    delay0); stage2: pair = p_hi + p_lo; stage3: running page sum (CURR
    feedback; zero-lane reset on step uops); stage4: even-page sum frozen via
    BYPASS(CURR) self-hold during odd pages, odd running sum riding delay0;
    write (even|odd) as one 32-bit pair at each odd page's subdim-last."""
    def mk(kind):
        u = UopConfig()
        u.enable_input(InpSel.SRC_0, 1)
        u.enable_input(InpSel.SRC_1, 2)
        u.enable_input(InpSel.ZERO, 3)
        u.enable_input(InpSel.SRC_0_HI, 4)
        u.enable_input(InpSel.SRC_1_HI, 5)
        u.datapath_config[0] = (
            UopDpConfig()
            .enable_alu(AluOp.MULTIPLY, AluInp.PREV_DELAY_0, AluInp.PREV_DELAY_1)
            .pass_through_delay(2, 3, 4)
        )
        u.datapath_config[1] = (
            UopDpConfig()
            .enable_alu(AluOp.MULTIPLY, AluInp.PREV_DELAY_3, AluInp.PREV_DELAY_4)
            .enable_delay_from_src(DelayInp.PREV_ALU_OUT, 0)
            .pass_through_delay(2)
        )
        u.datapath_config[2] = (
            UopDpConfig()
            .enable_alu(AluOp.ADD, AluInp.PREV_ALU_OUT, AluInp.PREV_DELAY_0)
            .pass_through_delay(2)
        )
        if kind == "seed":
            u.datapath_config[3] = UopDpConfig().enable_alu(
                AluOp.BYPASS, AluInp.PREV_DELAY_2, AluInp.PREV_DELAY_2
            )
        elif kind in ("stepO", "stepE"):
            u.datapath_config[3] = UopDpConfig().enable_alu(
                AluOp.ADD, AluInp.PREV_DELAY_2, AluInp.PREV_ALU_OUT
            )
        else:  # steadyE / steadyO
            u.datapath_config[3] = UopDpConfig().enable_alu(
                AluOp.ADD, AluInp.CURR_ALU_OUT, AluInp.PREV_ALU_OUT
            )
        hold = kind in ("stepO", "steadyO")
        u.datapath_config[4] = (
            UopDpConfig()
            .enable_alu(
                AluOp.BYPASS,
                AluInp.CURR_ALU_OUT if hold else AluInp.PREV_ALU_OUT,
                AluInp.CURR_ALU_OUT if hold else AluInp.PREV_ALU_OUT,
            )
            .enable_delay_from_src(DelayInp.PREV_ALU_OUT, 0)
        )
        for k in range(5, 8):
            u.datapath_config[k] = (
                UopDpConfig().pass_through_alu().pass_through_delay(0)
            )
        if kind != "seed":
            u.require_inp0 = 1
            u.require_inp1 = 1
        return u

    seed = mk("seed")
    seed.repeat_count = 1
    seed.trigger = (Trigger.COUNT, Trigger.NONE, Trigger.NONE)
    seed.next_uop = (1, 0, 0)

    steadyE = mk("steadyE")
    steadyE.trigger = (Trigger.SRC_TENSOR_DONE, Trigger.SUB_DIM_DONE, Trigger.NONE)
    steadyE.next_uop = (0, 2, 0)

    stepO = mk("stepO")
    stepO.repeat_count = 1
    stepO.trigger = (Trigger.SRC_TENSOR_DONE, Trigger.COUNT, Trigger.NONE)
    stepO.next_uop = (0, 3, 0)

    steadyO = mk("steadyO")
    steadyO.trigger = (Trigger.SRC_TENSOR_DONE, Trigger.SUB_DIM_DONE, Trigger.NONE)
    steadyO.next_uop = (0, 4, 0)
    steadyO.out_last_subdim_enable = 1
    steadyO.enable_output(OutSel.ALU_OUT, OutPath.WR0_LO)
    steadyO.enable_output(OutSel.DELAY_0, OutPath.WR0_HI)

    stepE = mk("stepE")
    stepE.repeat_count = 1
    stepE.trigger = (Trigger.SRC_TENSOR_DONE, Trigger.COUNT, Trigger.NONE)
    stepE.next_uop = (0, 1, 0)

    uops = [seed, steadyE, stepO, steadyO, stepE]
    for u in uops:
        u.validate("v3")
    return uops


def _segsum_parity_1x_uops():
    """Correct 1x fallback (one sum written per page), shaped as 5 states to
    mirror the 2x program (table-gen requires matching state counts)."""
    def mk(kind):
        u = UopConfig()
        u.enable_input(InpSel.SRC_0, 1)
        u.enable_input(InpSel.SRC_1, 2)
        u.enable_input(InpSel.ZERO, 3)
        u.datapath_config[0] = (
            UopDpConfig()
            .enable_alu(AluOp.MULTIPLY, AluInp.PREV_DELAY_0, AluInp.PREV_DELAY_1)
            .pass_through_delay(2)
        )
        if kind == "seed":
            u.datapath_config[1] = UopDpConfig().enable_alu(
                AluOp.BYPASS, AluInp.PREV_DELAY_2, AluInp.PREV_DELAY_2
            )
        elif kind.startswith("step"):
            u.datapath_config[1] = UopDpConfig().enable_alu(
                AluOp.ADD, AluInp.PREV_DELAY_2, AluInp.PREV_ALU_OUT
            )
        else:
            u.datapath_config[1] = UopDpConfig().enable_alu(
                AluOp.ADD, AluInp.CURR_ALU_OUT, AluInp.PREV_ALU_OUT
            )
        for k in range(2, 8):
            u.datapath_config[k] = UopDpConfig().pass_through_alu()
        if kind != "seed":
            u.require_inp0 = 1
            u.require_inp1 = 1
            u.out_last_subdim_enable = 1
            u.enable_output(OutSel.ALU_OUT, OutPath.WR0_LO)
        return u

    seed = mk("seed")
    seed.repeat_count = 1
    seed.trigger = (Trigger.COUNT, Trigger.NONE, Trigger.NONE)
    seed.next_uop = (1, 0, 0)

    steadyE = mk("steadyE")
    steadyE.trigger = (Trigger.SRC_TENSOR_DONE, Trigger.SUB_DIM_DONE, Trigger.NONE)
    steadyE.next_uop = (0, 2, 0)

    stepO = mk("stepO")
    stepO.repeat_count = 1
    stepO.trigger = (Trigger.SRC_TENSOR_DONE, Trigger.COUNT, Trigger.NONE)
    stepO.next_uop = (0, 3, 0)

    steadyO = mk("steadyO")
    steadyO.trigger = (Trigger.SRC_TENSOR_DONE, Trigger.SUB_DIM_DONE, Trigger.NONE)
    steadyO.next_uop = (0, 4, 0)

    stepE = mk("stepE")
    stepE.repeat_count = 1
    stepE.trigger = (Trigger.SRC_TENSOR_DONE, Trigger.COUNT, Trigger.NONE)
    stepE.next_uop = (0, 1, 0)

    uops = [seed, steadyE, stepO, steadyO, stepE]
    for u in uops:
        u.validate("v3")
    return uops


def _tpose_uops():
    """Bare 32x32-block transpose: TRANSPOSE front-end + pass-through body."""
    u = UopConfig()
    u.enable_input(InpSel.SRC_0, 1)
    u.datapath_config[0] = UopDpConfig().enable_alu(
        AluOp.BYPASS, AluInp.PREV_DELAY_0, AluInp.PREV_DELAY_0
    )
    for k in range(1, 8):
        u.datapath_config[k] = UopDpConfig().pass_through_alu()
    u.require_inp0 = 1
    u.trigger = (Trigger.SRC_TENSOR_DONE, Trigger.NONE, Trigger.NONE)
    u.next_uop = (0, 0, 0)
    u.enable_output(OutSel.ALU_OUT, OutPath.WR0_LO)
    u.validate("v3")
    return [u]


def _get_ops():
    if "segsum" in _cache:
        return _cache["segsum"], _cache["tpose"]

    def _ref(in0, in1, s0, s1, imm2):
        return (np.asarray(in0, np.float32) * np.asarray(in1, np.float32)).sum(-1)

    spec = Spec(body=scan(AluOp.ADD, Src0 * Src1), reference=_ref)
    segsum = _register(
        "SEGSUMP2X_ANT", spec, True, _segsum_parity_1x_uops(),
        uops_2x=_segsum_parity_2x_uops(), op_cfg=OpConfig(),
    )
    spec_tp = Spec(body=Src0 + Src0, reference=lambda in0, s0, s1, imm2: in0)
    tpose = _register(
        "TPOSE_ANT", spec_tp, False, _tpose_uops(),
        op_cfg=OpConfig(transpose_mode=TransposeMode.TRANSPOSE), rd1_en=False,
    )
    _cache["segsum"] = segsum
    _cache["tpose"] = tpose
    return segsum, tpose


def _emit_dve(nc, op, *, out, in0, in1=None, perf_max=0):
    """Mirror of bass.Vector._custom_dve, plus the perf_max (byte-36[7:6])
    field that unlocks the 2x_1P table slot."""
    v = nc.vector
    if op.name not in v.bass.m.ant_custom_dve_ops:
        v.bass.m.ant_custom_dve_ops = sorted(
            {*v.bass.m.ant_custom_dve_ops, op.name}
        )
    compiled = op.compile("v3")
    opt = not op.subdim
    in1_elementwise = len(in1.shape) > 2 if in1 is not None else False
    shape = (
        bass_isa.CustomDveShape.STT if in1_elementwise
        else bass_isa.CustomDveShape.TTSS
    )
    isa_opcode = v.bass.isa.Opcode[
        f"NEURON_ISA_TPB_OPCODE_CUSTOM_DVE_ANT_{shape.slot()}"
    ].value
    zero = mybir.ImmediateValue(dtype=mybir.dt.float32, value=0.0)
    ins = [v.lower_ap(in0, for_isa=True, opt=opt)]
    if in1 is not None:
        ins.append(v.lower_ap(in1, for_isa=True, opt=opt))
    ins += [zero, zero]
    outs = [v.lower_ap(out, for_isa=True, opt=opt)]
    return v.add_instruction(
        bass_isa.InstCustomDveAnt(
            name=v.bass.get_next_instruction_name(),
            op_name=op.name,
            rd1_en=compiled.rd1_en,
            subdim=0x02 if op.subdim else 0,
            imm2=0.0,
            shape=shape,
            row=compiled.opcode,
            isa_opcode=isa_opcode,
            ins=ins,
            outs=outs,
            perf_max=perf_max,
        )
    )


def _build_program() -> bass.Bass:
    from contextlib import ExitStack

    segsum, tpose = _get_ops()
    nc = bacc.Bacc("TRN2", target_bir_lowering=False)
    # fwd: steps 2..63 in [s,b,v,w,u]; bwd: steps 125..64 in [s,b,v,u,w]
    scf = nc.dram_tensor("scf", [NSLOT, BL, T, T, T], FP8, kind="ExternalInput")
    scb = nc.dram_tensor("scb", [NSLOT, BL, T, T, T], FP8, kind="ExternalInput")
    d2in = nc.dram_tensor("init_d2", [P, T], BF16, kind="ExternalInput")
    g126in = nc.dram_tensor("init_g126", [P, T], BF16, kind="ExternalInput")
    dout = nc.dram_tensor("dfin", [P, 2 * T], BF16, kind="ExternalOutput")

    with tile.TileContext(nc) as tc, ExitStack() as ctx:
        rawf = ctx.enter_context(tc.tile_pool(name="rawf", bufs=3))
        efp = ctx.enter_context(tc.tile_pool(name="efp", bufs=3))
        spool = ctx.enter_context(tc.tile_pool(name="spool", bufs=3))
        xpool = ctx.enter_context(tc.tile_pool(name="xpool", bufs=4))
        small = ctx.enter_context(tc.tile_pool(name="small", bufs=1))

        cbias = small.tile([P, 1], F32)
        nc.vector.memset(cbias[...], -C_OFF)
        # tiny warm-up activation: forces ACT_TABLE_LOAD at t~8us instead of
        # behind the first chunk's DMA-completion wait
        warm = small.tile([P, 1], F32)
        nc.scalar.activation(
            out=warm[...], in_=cbias[...],
            func=mybir.ActivationFunctionType.Exp,
        )

        def chunk_dma(eng, dst, dram, s0, ch):
            eng.dma_start(
                out=dst[...],
                in_=bass.AP(
                    tensor=dram[...].tensor,
                    offset=s0 * SB,
                    ap=[[T * T, P], [SB, ch], [T, T], [1, T]],
                ),
            )

        # chunk-0 DMAs first: the first exp gates the whole pipeline.
        # Raw/e tiles interleave the two chains per step: [P, ch, 2, T, T]
        # (fwd at [:, :, 0], bwd at [:, :, 1]) -> ONE activation per chunk.
        def chunk_tiles(ch):
            rw = rawf.tile([P, ch, 2, T, T], FP8)
            return rw

        rw0 = chunk_tiles(CHUNKS[0])
        chunk_dma(nc.sync, rw0[:, :, 0], scf, 0, CHUNKS[0])
        chunk_dma(nc.sync, rw0[:, :, 1], scb, 0, CHUNKS[0])

        d0t = small.tile([P, T], BF16)
        d_cur = d0t[...]
        nc.sync.dma_start(out=d_cur, in_=d2in[...])
        g0t = small.tile([P, T], BF16)
        g_cur = g0t[...]
        nc.sync.dma_start(out=g_cur, in_=g126in[...])

        s0 = 0
        for ci, ch in enumerate(CHUNKS):
            if ci == 0:
                rw = rw0
            else:
                rw = chunk_tiles(ch)
                chunk_dma(nc.sync, rw[:, :, 0], scf, s0, ch)
                chunk_dma(nc.sync, rw[:, :, 1], scb, s0, ch)
            ee = efp.tile([P, ch, 2, T, T], BF16)
            if ci == 0:
                # split fwd/bwd so the first fwd step unblocks without
                # waiting for the bwd chunk's DMA
                nc.scalar.activation(
                    out=ee[:, :, 0], in_=rw[:, :, 0],
                    func=mybir.ActivationFunctionType.Exp, bias=cbias[...],
                )
                nc.scalar.activation(
                    out=ee[:, :, 1], in_=rw[:, :, 1],
                    func=mybir.ActivationFunctionType.Exp, bias=cbias[...],
                )
            else:
                nc.scalar.activation(
                    out=ee[...], in_=rw[...],
                    func=mybir.ActivationFunctionType.Exp, bias=cbias[...],
                )
            for j in range(ch):
                last = (s0 + j) == NSLOT - 1
                xf = xpool.tile([P, T], BF16)
                _emit_dve(nc, tpose, out=xf[...], in0=d_cur)
                xb = xpool.tile([P, T], BF16)
                _emit_dve(nc, tpose, out=xb[...], in0=g_cur)
                if last:
                    fin = small.tile([P, 2 * T], BF16)
                    d_nxt, g_nxt = fin[:, 0:T], fin[:, T:2 * T]
                else:
                    dnt = spool.tile([P, T], BF16)
                    gnt = spool.tile([P, T], BF16)
                    d_nxt, g_nxt = dnt[...], gnt[...]
                _emit_dve(
                    nc, segsum, out=d_nxt,
                    in0=xf[...].unsqueeze(1).broadcast_to([P, T, T]),
                    in1=ee[:, j, 0], perf_max=1,
                )
                _emit_dve(
                    nc, segsum, out=g_nxt,
                    in0=xb[...].unsqueeze(1).broadcast_to([P, T, T]),
                    in1=ee[:, j, 1], perf_max=1,
                )
                d_cur, g_cur = d_nxt, g_nxt
            s0 += ch
        nc.sync.dma_start(out=dout[...], in_=fin[...])
    nc.compile()
    return nc


def _get_program() -> bass.Bass:
    if "nc" not in _cache:
        _cache["nc"] = _build_program()
    return _cache["nc"]


def kernel(scores, target, mask=None, **_unused):
    import ml_dtypes

    BH = ml_dtypes.bfloat16
    scores = np.asarray(scores, dtype=np.float32)
    target = np.asarray(target)

    F8 = ml_dtypes.float8_e4m3
    # fwd E-layout [s,b,v,w,u] for steps 2..63; bwd [s,b,v,u,w] for steps
    # 125..64 (slot k = step 125-k)
    scf = np.ascontiguousarray(
        scores[2:M].transpose(0, 1, 3, 4, 2)
    ).astype(F8)
    scb = np.ascontiguousarray(
        scores[M:126].transpose(0, 1, 3, 2, 4)[::-1]
    ).astype(F8)

    # seeds
    p1 = scores[0, :, START, START, :]                    # (B, i)
    part2 = p1[:, :, None] + scores[1, :, START, :, :]    # (B, i, j)
    d2 = np.exp(part2 - C_OFF).astype(BH)                 # stored [(b,i), j]
    g127_j = np.exp(scores[127, :, :, END, END] - C_OFF)  # (B, j)
    g126 = np.exp(scores[126, :, :, :, END] - C_OFF) * g127_j[:, None, :]
    g126 = np.ascontiguousarray(g126.transpose(0, 2, 1)).astype(BH)  # [(b,j), i]

    nc = _get_program()
    in_maps = []
    for core in range(NCORES):
        bs = slice(core * BL, (core + 1) * BL)
        in_maps.append({
            "scf": np.ascontiguousarray(scf[:, bs]),
            "scb": np.ascontiguousarray(scb[:, bs]),
            "init_d2": np.ascontiguousarray(d2[bs]).reshape(P, T),
            "init_g126": np.ascontiguousarray(g126[bs]).reshape(P, T),
        })

    res = bass_utils.run_bass_kernel_spmd(nc, in_maps, core_ids=list(range(NCORES)))
    global LAST_RESULT
    LAST_RESULT = res

    total_z = 0.0
    for core in range(NCORES):
        fin = np.asarray(res.results[core]["dfin"], np.float32).astype(np.float64)
        D = fin[:, 0:T].reshape(BL, T, T)
        G = fin[:, T:2 * T].reshape(BL, T, T)
        z_be = np.einsum("bij,bji->b", D, G)
        total_z += (np.log(z_be) + (S - 1) * C_OFF).sum()

    flat = scores.reshape(S, B, -1)
    tg = np.take_along_axis(flat, target.reshape(S, B, 1).astype(np.int64), axis=2)
    tg_energy = tg.astype(np.float64).sum()

    return np.asarray((total_z - tg_energy) / B, dtype=np.float32)
